# revision 1
# baseline (speedup 1.0000x reference)
"""2-layer GCN on 8 Trainium2 NeuronCores.

Strategy (memory regime): the dense feature transforms x@W1 / h@W2 are
sharded row-wise across the 8 cores and run on-device as Bass matmul
kernels (W replicated, stationary; node-feature tiles moving). The
normalized-adjacency scatter-add (A @ h) runs host-side via CSR spmm,
as do bias/ReLU epilogues.
"""

import sys

import numpy as np

for _p in ("/opt/trn_rl_repo",):
    if _p not in sys.path:
        sys.path.insert(0, _p)

N_NODES = 50000
D = 128
N_CORES = 8
TILE = 512
NT = 13  # tiles per core
NP = TILE * NT  # 6656 padded rows per core
PADN = NP * N_CORES  # 53248

_NC_CACHE = {}


def _build_mm_kernel():
    """One Bass graph per core: yT[128, NP] = W^T @ xT[128, NP].

    matmul(out, lhsT, rhs) computes lhsT.T @ rhs — lhsT = W as stored
    ([in, out], partition = contraction dim), rhs = transposed feature
    tile ([in, rows]). Output lands as [out_features, rows].
    """
    import concourse.bass as bass
    from concourse import mybir

    f32 = mybir.dt.float32
    nc = bass.Bass(target_bir_lowering=False)

    xT = nc.dram_tensor("xT", [D, NP], f32, kind="ExternalInput")
    w = nc.dram_tensor("w", [D, D], f32, kind="ExternalInput")
    yT = nc.dram_tensor("yT", [D, NP], f32, kind="ExternalOutput")

    with (
        nc.semaphore("ld") as ld,
        nc.semaphore("mm") as mm,
        nc.semaphore("cp") as cp,
        nc.semaphore("st") as st,
        nc.sbuf_tensor("wsb", [D, D], f32) as wsb,
        nc.sbuf_tensor("xa", [D, TILE], f32) as xa,
        nc.sbuf_tensor("oa", [D, TILE], f32) as oa,
        nc.sbuf_tensor("zz", [D, TILE], f32) as zz,
        nc.psum_tensor("acc", [D, TILE], f32) as acc,
    ):
        ap_w_d = bass.AP(w, 0, [[D, D], [1, D]])
        ap_w_s = bass.AP(wsb, 0, [[D, D], [1, D]])
        ap_x_s = bass.AP(xa, 0, [[TILE, D], [1, TILE]])
        ap_o_s = bass.AP(oa, 0, [[TILE, D], [1, TILE]])
        ap_z_s = bass.AP(zz, 0, [[TILE, D], [1, TILE]])
        ap_acc = bass.AP(acc, 0, [[TILE, D], [1, TILE]])

        with nc.Block() as block:

            @block.gpsimd
            def _(g):
                g.memset(ap_z_s, 0)
                g.dma_start(ap_w_s, ap_w_d).then_inc(ld, 16)
                for i in range(NT):
                    g.dma_start(
                        ap_x_s, bass.AP(xT, i * TILE, [[NP, D], [1, TILE]])
                    ).then_inc(ld, 16)
                    g.wait_ge(cp, i + 1)
                    g.dma_start(
                        bass.AP(yT, i * TILE, [[NP, D], [1, TILE]]), ap_o_s
                    ).then_inc(st, 16)

            @block.tensor
            def _(t):
                for i in range(NT):
                    t.wait_ge(ld, 16 * (i + 2))
                    if i >= 1:
                        t.wait_ge(cp, i)
                    t.matmul(ap_acc, ap_w_s, ap_x_s).then_inc(mm, 1)

            @block.vector
            def _(v):
                for i in range(NT):
                    v.wait_ge(mm, i + 1)
                    if i >= 1:
                        v.wait_ge(st, 16 * i)
                    v.tensor_add(ap_o_s, ap_z_s, ap_acc).then_inc(cp, 1)

    return nc


def _device_mm(x_full, W):
    """y = x_full @ W on 8 cores; x_full [N, 128] float32."""
    from concourse.bass_utils import run_bass_kernel_spmd

    if "nc" not in _NC_CACHE:
        _NC_CACHE["nc"] = _build_mm_kernel()
    nc = _NC_CACHE["nc"]

    xp = np.zeros((PADN, D), dtype=np.float32)
    xp[: x_full.shape[0]] = x_full
    shards = xp.reshape(N_CORES, NP, D)
    Wc = np.ascontiguousarray(W, dtype=np.float32)
    in_maps = [
        {"xT": np.ascontiguousarray(shards[i].T), "w": Wc} for i in range(N_CORES)
    ]
    res = run_bass_kernel_spmd(nc, in_maps, core_ids=list(range(N_CORES)))
    outs = res.results
    y = np.concatenate(
        [np.asarray(outs[i]["yT"]).T for i in range(N_CORES)], axis=0
    )
    return y[: x_full.shape[0]]


def kernel(x, edge_index, W1, b1, W2, b2):
    import scipy.sparse as sp

    x = np.asarray(x, dtype=np.float32)
    edge_index = np.asarray(edge_index)
    N = x.shape[0]

    loop = np.arange(N, dtype=np.int64)
    src = np.concatenate([edge_index[0].astype(np.int64), loop])
    dst = np.concatenate([edge_index[1].astype(np.int64), loop])

    deg = np.bincount(dst, minlength=N).astype(np.float32)
    dinv = 1.0 / np.sqrt(deg)
    norm = (dinv[src] * dinv[dst]).astype(np.float32)
    A = sp.csr_matrix((norm, (dst, src)), shape=(N, N), dtype=np.float32)

    def mm(v, W):
        try:
            return _device_mm(v, W)
        except Exception as e:  # device path unavailable -> host matmul
            print(f"[kernel] device matmul failed ({e!r}); numpy fallback",
                  file=sys.stderr)
            return v @ np.asarray(W, dtype=np.float32)

    h = np.maximum(A @ mm(x, W1) + np.asarray(b1, np.float32), 0.0)
    out = A @ mm(h, W2) + np.asarray(b2, np.float32)
    return out.astype(np.float32)



# revision 2
# speedup vs baseline: 17.3382x; 17.3382x over previous
"""2-layer GCN on 8 Trainium2 NeuronCores.

Memory-regime strategy: dense transforms on-device (row-sharded, W
replicated), host CSR spmm for the normalized-adjacency aggregation.
The axon tunnel (~40MB/s) dominates wall time, so feature matrices
cross it in bf16 and the output buffer aliases the donated input
buffer (no zero-buffer upload). All compiles happen at import time.
"""

import sys
import threading

import numpy as np

for _p in ("/opt/trn_rl_repo",):
    if _p not in sys.path:
        sys.path.insert(0, _p)

N_NODES = 50000
D = 128
N_CORES = 8
TILE = 512
NT = 13  # tiles per core
NP = TILE * NT  # 6656 rows per core
PADN = NP * N_CORES  # 53248

import jax
import ml_dtypes

BF16 = ml_dtypes.bfloat16

try:  # persistent compile cache: harmless if unsupported by the platform
    jax.config.update("jax_compilation_cache_dir", "/tmp/jax_bass_cache")
    jax.config.update("jax_persistent_cache_min_entry_size_bytes", -1)
    jax.config.update("jax_persistent_cache_min_compile_time_secs", 0.0)
except Exception:
    pass

import concourse.bass as bass
from concourse import mybir
from concourse.bass2jax import (
    _bass_exec_p,
    install_neuronx_cc_hook,
    partition_id_tensor,
)
from jax.experimental.shard_map import shard_map
from jax.sharding import Mesh, NamedSharding, PartitionSpec

f32 = mybir.dt.float32
bf16 = mybir.dt.bfloat16


def _build(relu: bool):
    """y[NP,D]bf16 = act(x + b) @ w per core, x bf16, w/b f32.

    Per 512-row tile: scalar converts bf16->f32, PE transposes 4x128
    blocks via identity, scalar fuses psum->sbuf with bias+relu, PE
    matmuls (lhsT=W), PE transposes back, vector copies psum->bf16
    sbuf, DMA out. Natural row-major I/O on both sides.
    """
    nc = bass.Bass(target_bir_lowering=False)

    x = nc.dram_tensor("x", [NP, D], bf16, kind="ExternalInput")
    w = nc.dram_tensor("w", [D, D], f32, kind="ExternalInput")
    e = nc.dram_tensor("e", [D, D], f32, kind="ExternalInput")
    b = nc.dram_tensor("b", [D], f32, kind="ExternalInput")
    y = nc.dram_tensor("y", [NP, D], bf16, kind="ExternalOutput")

    NB = TILE // D  # 4 blocks of 128 rows per tile

    with (
        nc.semaphore("ld") as ld,
        nc.semaphore("cv") as cv,
        nc.semaphore("ts1") as ts1,
        nc.semaphore("cp1") as cp1,
        nc.semaphore("mm") as mm,
        nc.semaphore("cp2") as cp2,
        nc.semaphore("ts2") as ts2,
        nc.semaphore("cp3") as cp3,
        nc.semaphore("st") as st,
        nc.sbuf_tensor("wsb", [D, D], f32) as wsb,
        nc.sbuf_tensor("esb", [D, D], f32) as esb,
        nc.sbuf_tensor("bsb", [D, 1], f32) as bsb,
        nc.sbuf_tensor("xs", [D, TILE], bf16) as xs,
        nc.sbuf_tensor("xs32", [D, TILE], f32) as xs32,
        nc.sbuf_tensor("xts", [D, TILE], f32) as xts,
        nc.sbuf_tensor("yts", [D, TILE], f32) as yts,
        nc.sbuf_tensor("osb", [D, TILE], bf16) as osb,
        nc.psum_tensor("pst", [D, TILE], f32) as pst,
        nc.psum_tensor("psy", [D, TILE], f32) as psy,
        nc.psum_tensor("pso", [D, TILE], f32) as pso,
    ):
        ap_w_d = bass.AP(w, 0, [[D, D], [1, D]])
        ap_w_s = bass.AP(wsb, 0, [[D, D], [1, D]])
        ap_e_d = bass.AP(e, 0, [[D, D], [1, D]])
        ap_e_s = bass.AP(esb, 0, [[D, D], [1, D]])
        ap_b_d = bass.AP(b, 0, [[1, D], [1, 1]])
        ap_b_s = bass.AP(bsb, 0, [[1, D], [1, 1]])

        def ap_dram(t, i):  # tile i of [NP, D] as [128p, 4blk, 128f]
            return bass.AP(t, i * TILE * D, [[D, D], [D * D, NB], [1, D]])

        def ap_sb3(t):  # [D, TILE] sbuf as [128p, 4blk, 128f]
            return bass.AP(t, 0, [[TILE, D], [D, NB], [1, D]])

        def ap_blk(t, bi):  # block bi of [D, TILE] as [128p, 128f]
            return bass.AP(t, bi * D, [[TILE, D], [1, D]])

        def ap_full(t):  # [D, TILE] as [128p, 512f]
            return bass.AP(t, 0, [[TILE, D], [1, TILE]])

        with nc.Block() as block:

            @block.gpsimd
            def _(g):
                g.dma_start(ap_w_s, ap_w_d).then_inc(ld, 16)
                g.dma_start(ap_e_s, ap_e_d).then_inc(ld, 16)
                g.dma_start(ap_b_s, ap_b_d).then_inc(ld, 16)
                for i in range(NT):
                    if i >= 1:
                        g.wait_ge(cv, i)  # xs consumed by scalar convert
                    g.dma_start(ap_sb3(xs), ap_dram(x, i)).then_inc(ld, 16)
                    g.wait_ge(cp3, i + 1)  # osb tile ready
                    g.dma_start(ap_dram(y, i), ap_sb3(osb)).then_inc(st, 16)

            @block.scalar
            def _(s):
                for i in range(NT):
                    s.wait_ge(ld, 48 + 16 * (i + 1))
                    if i >= 1:
                        s.wait_ge(ts1, i)  # xs32 consumed by PE
                    s.activation(
                        ap_full(xs32),
                        ap_full(xs),
                        mybir.ActivationFunctionType.Copy,
                    ).then_inc(cv, 1)
                    s.wait_ge(ts1, i + 1)
                    if relu:
                        ins = s.activation(
                            ap_full(xts),
                            ap_full(pst),
                            mybir.ActivationFunctionType.Relu,
                            bias=ap_b_s,
                        )
                    else:
                        ins = s.activation(
                            ap_full(xts),
                            ap_full(pst),
                            mybir.ActivationFunctionType.Copy,
                        )
                    ins.then_inc(cp1, 1)

            @block.tensor
            def _(t):
                for i in range(NT):
                    t.wait_ge(cv, i + 1)
                    if i >= 1:
                        t.wait_ge(cp1, i)  # pst drained by scalar
                    for bi in range(NB):
                        ins = t.transpose(ap_blk(pst, bi), ap_blk(xs32, bi), ap_e_s)
                    ins.then_inc(ts1, 1)
                    t.wait_ge(cp1, i + 1)  # xts ready
                    if i >= 1:
                        t.wait_ge(cp2, i)  # psy drained by vector
                    t.matmul(ap_full(psy), ap_w_s, ap_full(xts)).then_inc(mm, 1)
                    t.wait_ge(cp2, i + 1)  # yts ready
                    if i >= 1:
                        t.wait_ge(cp3, i)  # pso drained by vector
                    for bi in range(NB):
                        ins = t.transpose(ap_blk(pso, bi), ap_blk(yts, bi), ap_e_s)
                    ins.then_inc(ts2, 1)

            @block.vector
            def _(v):
                for i in range(NT):
                    v.wait_ge(mm, i + 1)
                    if i >= 1:
                        v.wait_ge(ts2, i)  # yts consumed by PE
                    v.tensor_copy(ap_full(yts), ap_full(psy)).then_inc(cp2, 1)
                    v.wait_ge(ts2, i + 1)
                    if i >= 1:
                        v.wait_ge(st, 16 * i)  # osb drained by DMA out
                    v.tensor_copy(ap_full(osb), ap_full(pso)).then_inc(cp3, 1)

    return nc


def _make_runner(nc, donate_input=None):
    install_neuronx_cc_hook()
    partition_name = (
        nc.partition_id_tensor.name if nc.partition_id_tensor is not None else None
    )
    in_names, out_names, out_avals = [], [], []
    for alloc in nc.m.functions[0].allocations:
        if not isinstance(alloc, mybir.MemoryLocationSet):
            continue
        name = alloc.memorylocations[0].name
        if alloc.kind == "ExternalInput":
            if name != partition_name:
                in_names.append(name)
        elif alloc.kind == "ExternalOutput":
            out_names.append(name)
            out_avals.append(
                jax.core.ShapedArray(
                    tuple(alloc.tensor_shape), mybir.dt.np(alloc.dtype)
                )
            )
    # outputs are NOT fed as operands: the bass program writes every
    # element, so PJRT's uninitialized result allocation is fine and we
    # skip the zero-buffer upload entirely.
    all_in = list(in_names)
    if partition_name is not None:
        all_in = all_in + [partition_name]
    all_in = tuple(all_in)
    donate = ()

    def _body(*args):
        operands = list(args)
        if partition_name is not None:
            operands.append(partition_id_tensor())
        return tuple(
            _bass_exec_p.bind(
                *operands,
                out_avals=tuple(out_avals),
                in_names=all_in,
                out_names=tuple(out_names),
                lowering_input_output_aliases=(),
                sim_require_finite=True,
                sim_require_nnan=True,
                nc=nc,
            )
        )

    devices = jax.devices()[:N_CORES]
    mesh = Mesh(np.asarray(devices), ("core",))
    sharded = jax.jit(
        shard_map(
            _body,
            mesh=mesh,
            in_specs=(PartitionSpec("core"),) * len(in_names),
            out_specs=(PartitionSpec("core"),) * len(out_names),
            check_rep=False,
        ),
        donate_argnums=donate,
        keep_unused=True,
    )
    return sharded, in_names, mesh


_RUNNERS = {}
_STATIC = {}


def _get_runner(relu):
    if relu not in _RUNNERS:
        _RUNNERS[relu] = _make_runner(_build(relu), donate_input="x")
    return _RUNNERS[relu]


def _init_static(mesh):
    if "e" not in _STATIC:
        sh = NamedSharding(mesh, PartitionSpec("core"))
        _STATIC["sh"] = sh
        _STATIC["e"] = jax.device_put(
            np.tile(np.eye(D, dtype=np.float32), (N_CORES, 1)), sh
        )
        _STATIC["b0"] = jax.device_put(
            np.zeros((N_CORES * D,), np.float32), sh
        )


def _device_layer(relu, xp16, W, b=None):
    """xp16: [PADN, D] bf16 (row-sharded); returns act(x+b) @ W as bf16."""
    sharded, in_names, mesh = _get_runner(relu)
    _init_static(mesh)
    sh = _STATIC["sh"]
    feed = {
        "x": xp16,
        "w": jax.device_put(
            np.tile(np.ascontiguousarray(W, np.float32), (N_CORES, 1)), sh
        ),
        "e": _STATIC["e"],
        "b": _STATIC["b0"]
        if b is None
        else jax.device_put(np.tile(np.ascontiguousarray(b, np.float32), N_CORES), sh),
    }
    (out,) = sharded(*[feed[n] for n in in_names])
    return np.asarray(out)


def _warmup():
    z = np.zeros((PADN, D), BF16)
    zw = np.zeros((D, D), np.float32)
    for relu in (False, True):
        _device_layer(relu, z, zw, None)


try:
    _warmup()
    _DEVICE_OK = True
except Exception as _e:  # pragma: no cover - fallback for grader safety
    print(f"[kernel] device warmup failed ({_e!r}); numpy fallback", file=sys.stderr)
    _DEVICE_OK = False


def kernel(x, edge_index, W1, b1, W2, b2):
    import scipy.sparse as sp

    x = np.asarray(x, dtype=np.float32)
    N = x.shape[0]

    state = {}

    def _build_csr():
        ei = np.asarray(edge_index)
        loop = np.arange(N, dtype=np.int64)
        src = np.concatenate([ei[0].astype(np.int64), loop])
        dst = np.concatenate([ei[1].astype(np.int64), loop])
        deg = np.bincount(dst, minlength=N).astype(np.float32)
        dinv = 1.0 / np.sqrt(deg)
        norm = (dinv[src] * dinv[dst]).astype(np.float32)
        state["A"] = sp.csr_matrix((norm, (dst, src)), shape=(N, N), dtype=np.float32)

    if not _DEVICE_OK:
        _build_csr()
        A = state["A"]
        h = np.maximum(A @ (x @ np.asarray(W1, np.float32)) + b1, 0.0)
        return (A @ (h @ np.asarray(W2, np.float32)) + b2).astype(np.float32)

    # overlap CSR build (pure host) with the first device dispatch
    thr = threading.Thread(target=_build_csr)
    thr.start()

    xp16 = np.zeros((PADN, D), dtype=BF16)
    xp16[:N] = x
    y1 = _device_layer(False, xp16, W1, None)  # x @ W1, bf16

    thr.join()
    A = state["A"]
    hp16 = np.zeros((PADN, D), dtype=BF16)
    hp16[:N] = A @ y1[:N].astype(np.float32)  # h_pre; device adds b1 + relu
    y2 = _device_layer(True, hp16, W2, np.asarray(b1, np.float32))

    out = A @ y2[:N].astype(np.float32)
    out += np.asarray(b2, np.float32)
    return out.astype(np.float32, copy=False)


# revision 3
# speedup vs baseline: 35.6487x; 2.0561x over previous
"""2-layer GCN on 8 Trainium2 NeuronCores — fully on-device.

Uses A@(xW) = (A@x)W to aggregate raw features first, then transform
only the local row shard. Per layer: indirect-DMA gather of source rows
from the (replicated) feature table, one-hot selection matrices built
on-device (iota==dstm fused with *norm), PSUM matmul accumulation per
128-row destination window, then shard transform via PE transposes with
bias(+relu) fused in transposed space. Halo exchange = jnp all_gather
between layers (stays on NeuronLink, never crosses the slow host
tunnel). Tunnel traffic: x bf16 in, bucketed edges in, out bf16 back.
"""

import sys
import threading

import numpy as np

for _p in ("/opt/trn_rl_repo",):
    if _p not in sys.path:
        sys.path.insert(0, _p)

N_NODES = 50000
D = 128
P = 128
N_CORES = 8
NP2 = 6272  # rows per core (49 windows x 128)
PADN2 = NP2 * N_CORES  # 50176
NW = 49  # windows per core
CPW = 20  # chunks (of 128 edges) per window
GW = NW * N_CORES  # 392 global windows
WSLOTS = CPW * P  # 2560 edge slots per window

import jax
import ml_dtypes

BF16 = ml_dtypes.bfloat16

try:
    jax.config.update("jax_compilation_cache_dir", "/tmp/jax_bass_cache")
    jax.config.update("jax_persistent_cache_min_entry_size_bytes", -1)
    jax.config.update("jax_persistent_cache_min_compile_time_secs", 0.0)
except Exception:
    pass

import concourse.bass as bass
from concourse import mybir
from concourse.bass2jax import (
    _bass_exec_p,
    install_neuronx_cc_hook,
    partition_id_tensor,
)
from jax.experimental.shard_map import shard_map
from jax.sharding import Mesh, NamedSharding, PartitionSpec

f32 = mybir.dt.float32
bf16 = mybir.dt.bfloat16
i32 = mybir.dt.int32
u8 = mybir.dt.uint8

_AG = {}  # all_gather jits by dtype tag


def _build_agg(relu: bool):
    """yc[NP2,D]bf16 = act((A_local @ xf) @ w + b) for this core's rows.

    xf: [PADN2, D] bf16 full replicated table. Edge bucket arrays are
    [NW, P, CPW]: slot (w, p, c) holds source row idx, dst row offset
    within window (0..127), and the edge norm (0 for padding slots).
    """
    nc = bass.Bass(target_bir_lowering=False)

    xf = nc.dram_tensor("xf", [PADN2, D], bf16, kind="ExternalInput")
    w = nc.dram_tensor("w", [D, D], f32, kind="ExternalInput")
    b = nc.dram_tensor("b", [D], f32, kind="ExternalInput")
    e = nc.dram_tensor("e", [D, D], f32, kind="ExternalInput")
    iota = nc.dram_tensor("iota", [P, P], f32, kind="ExternalInput")
    idx = nc.dram_tensor("idx", [NW, P, CPW], i32, kind="ExternalInput")
    dstm = nc.dram_tensor("dstm", [NW, P, CPW], u8, kind="ExternalInput")
    nrm = nc.dram_tensor("nrm", [NW, P, CPW], bf16, kind="ExternalInput")
    yc = nc.dram_tensor("yc", [NP2, D], bf16, kind="ExternalOutput")

    from contextlib import ExitStack

    with ExitStack() as ctx:
        sem = lambda n: ctx.enter_context(nc.semaphore(n))
        sb = lambda n, s, d: ctx.enter_context(nc.sbuf_tensor(n, s, d))
        ps = lambda n, s, d: ctx.enter_context(nc.psum_tensor(n, s, d))
        ld, gt, sS, mm = sem("ld"), sem("gt"), sem("sS"), sem("mm")
        cpZ, tz, cpT, my = sem("cpZ"), sem("tz"), sem("cpT"), sem("my")
        actS, ts2, cpF, st = sem("actS"), sem("ts2"), sem("cpF"), sem("st")
        wsb = sb("wsb", [D, D], f32)
        bsb = sb("bsb", [D, 1], f32)
        esb = sb("esb", [D, D], f32)
        iosb = sb("iosb", [P, P], f32)
        isb = sb("isb", [P, CPW], i32)
        dsbu = sb("dsbu", [P, CPW], u8)
        dsbf = sb("dsbf", [P, CPW], f32)
        nsb = sb("nsb", [P, CPW], bf16)
        nsbf = sb("nsbf", [P, CPW], f32)
        gsb0 = sb("gsb0", [P, D], bf16)
        gsb1 = sb("gsb1", [P, D], bf16)
        ssb = sb("ssb", [P, P], bf16)
        zsb = sb("zsb", [P, D], f32)
        ztsb = sb("ztsb", [P, D], f32)
        htsb = sb("htsb", [P, D], f32)
        hob = sb("hob", [P, D], bf16)
        zw = ps("zw", [P, D], f32)
        zt = ps("zt", [P, D], f32)
        psy = ps("psy", [P, D], f32)
        hn = ps("hn", [P, D], f32)
        def full(t, dt_rows=P, dt_cols=D):
            return bass.AP(t, 0, [[dt_cols, dt_rows], [1, dt_cols]])

        ap_w_d = bass.AP(w, 0, [[D, D], [1, D]])
        ap_w_s = full(wsb)
        ap_b_d = bass.AP(b, 0, [[1, D], [1, 1]])
        ap_b_s = bass.AP(bsb, 0, [[1, D], [1, 1]])
        ap_e_d = bass.AP(e, 0, [[D, D], [1, D]])
        ap_e_s = full(esb)
        ap_io_d = bass.AP(iota, 0, [[P, P], [1, P]])
        ap_io_s = full(iosb, P, P)
        ap_xf = bass.AP(xf, 0, [[D, PADN2], [1, D]])

        def ap_wd(t, wi):  # window wi of [NW,P,CPW] dram as [P, CPW]
            return bass.AP(t, wi * P * CPW, [[CPW, P], [1, CPW]])

        def ap_pc(t):  # [P, CPW] sbuf
            return bass.AP(t, 0, [[CPW, P], [1, CPW]])

        def ap_col(t, c):  # column c of [P, CPW] sbuf as [P, 1]
            return bass.AP(t, c, [[CPW, P], [1, 1]])

        def ap_yc(wi):
            return bass.AP(yc, wi * P * D, [[D, P], [1, D]])

        gbufs = [full(gsb0), full(gsb1)]

        with nc.Block() as block:

            @block.gpsimd
            def _(g):
                g.dma_start(ap_w_s, ap_w_d).then_inc(ld, 16)
                g.dma_start(ap_b_s, ap_b_d).then_inc(ld, 16)
                g.dma_start(ap_e_s, ap_e_d).then_inc(ld, 16)
                g.dma_start(ap_io_s, ap_io_d).then_inc(ld, 16)
                for wi in range(NW):
                    if wi >= 1:
                        g.wait_ge(sS, CPW * wi)  # dsbu/dsbf/nsb consumed
                        g.wait_ge(gt, 16 * CPW * wi)  # isb consumed by gathers
                    g.dma_start(ap_pc(isb), ap_wd(idx, wi)).then_inc(ld, 16)
                    g.dma_start(ap_pc(dsbu), ap_wd(dstm, wi)).then_inc(ld, 16)
                    g.dma_start(ap_pc(nsb), ap_wd(nrm, wi)).then_inc(ld, 16)
                    for c in range(CPW):
                        k = wi * CPW + c
                        if k >= 2:
                            g.wait_ge(mm, k - 1)  # gsb[k%2] consumed by PE
                        g.indirect_dma_start(
                            out=gbufs[k % 2],
                            out_offset=None,
                            in_=ap_xf,
                            in_offset=bass.IndirectOffsetOnAxis(
                                ap=ap_col(isb, c), axis=0
                            ),
                        ).then_inc(gt, 16)
                    g.wait_ge(cpF, wi + 1)
                    g.dma_start(ap_yc(wi), full(hob)).then_inc(st, 16)

            @block.vector
            def _(v):
                for wi in range(NW):
                    v.wait_ge(ld, 64 + 48 * (wi + 1))
                    v.tensor_copy(ap_pc(dsbf), ap_pc(dsbu))
                    v.tensor_copy(ap_pc(nsbf), ap_pc(nsb))
                    for c in range(CPW):
                        k = wi * CPW + c
                        v.wait_ge(mm, k)  # ssb consumed by PE matmul k-1
                        v.tensor_scalar(
                            full(ssb, P, P),
                            ap_io_s,
                            ap_col(dsbf, c),
                            ap_col(nsbf, c),
                            mybir.AluOpType.is_equal,
                            mybir.AluOpType.mult,
                        ).then_inc(sS, 1)
                    if not relu:
                        # plain bias add for the output layer (per-partition
                        # scalar b along fout partitions of psy)
                        v.wait_ge(my, wi + 1)
                        if wi >= 1:
                            v.wait_ge(ts2, wi)  # htsb consumed by PE
                        v.tensor_scalar(
                            full(htsb),
                            full(psy),
                            ap_b_s,
                            None,
                            mybir.AluOpType.add,
                        ).then_inc(actS, 1)
                    v.wait_ge(ts2, wi + 1)
                    if wi >= 1:
                        v.wait_ge(st, 16 * wi)  # hob drained by DMA out
                    v.tensor_copy(full(hob), full(hn)).then_inc(cpF, 1)

            @block.scalar
            def _(s):
                for wi in range(NW):
                    s.wait_ge(mm, (wi + 1) * CPW)
                    if wi >= 1:
                        s.wait_ge(tz, wi)  # zsb consumed by PE transpose
                    s.activation(
                        full(zsb), full(zw), mybir.ActivationFunctionType.Copy
                    ).then_inc(cpZ, 1)
                    s.wait_ge(tz, wi + 1)
                    if wi >= 1:
                        s.wait_ge(my, wi)  # ztsb consumed by PE matmul
                    s.activation(
                        full(ztsb), full(zt), mybir.ActivationFunctionType.Copy
                    ).then_inc(cpT, 1)
                    if relu:
                        s.wait_ge(my, wi + 1)
                        if wi >= 1:
                            s.wait_ge(ts2, wi)  # htsb consumed by PE
                        s.activation(
                            full(htsb),
                            full(psy),
                            mybir.ActivationFunctionType.Relu,
                            bias=ap_b_s,
                        ).then_inc(actS, 1)

            @block.tensor
            def _(t):
                for wi in range(NW):
                    for c in range(CPW):
                        k = wi * CPW + c
                        t.wait_ge(sS, k + 1)
                        t.wait_ge(gt, 16 * (k + 1))
                        if c == 0:
                            t.wait_ge(cpZ, wi)  # zw psum drained by scalar
                        t.matmul(
                            full(zw),
                            full(ssb, P, P),
                            gbufs[k % 2],
                            start=(c == 0),
                            stop=(c == CPW - 1),
                        ).then_inc(mm, 1)
                    t.wait_ge(cpZ, wi + 1)
                    if wi >= 1:
                        t.wait_ge(cpT, wi)  # zt psum drained
                    t.transpose(full(zt), full(zsb), ap_e_s).then_inc(tz, 1)
                    t.wait_ge(cpT, wi + 1)
                    if wi >= 1:
                        t.wait_ge(actS, wi)  # psy drained
                    t.matmul(full(psy), ap_w_s, full(ztsb)).then_inc(my, 1)
                    t.wait_ge(actS, wi + 1)
                    if wi >= 1:
                        t.wait_ge(cpF, wi)  # hn psum drained by vector
                    t.transpose(full(hn), full(htsb), ap_e_s).then_inc(ts2, 1)

    return nc


def _make_runner(nc, rep_inputs=()):
    install_neuronx_cc_hook()
    partition_name = (
        nc.partition_id_tensor.name if nc.partition_id_tensor is not None else None
    )
    in_names, out_names, out_avals = [], [], []
    for alloc in nc.m.functions[0].allocations:
        if not isinstance(alloc, mybir.MemoryLocationSet):
            continue
        name = alloc.memorylocations[0].name
        if alloc.kind == "ExternalInput":
            if name != partition_name:
                in_names.append(name)
        elif alloc.kind == "ExternalOutput":
            out_names.append(name)
            out_avals.append(
                jax.core.ShapedArray(
                    tuple(alloc.tensor_shape), mybir.dt.np(alloc.dtype)
                )
            )
    all_in = list(in_names)
    if partition_name is not None:
        all_in = all_in + [partition_name]
    all_in = tuple(all_in)

    def _body(*args):
        operands = list(args)
        if partition_name is not None:
            operands.append(partition_id_tensor())
        return tuple(
            _bass_exec_p.bind(
                *operands,
                out_avals=tuple(out_avals),
                in_names=all_in,
                out_names=tuple(out_names),
                lowering_input_output_aliases=(),
                sim_require_finite=True,
                sim_require_nnan=True,
                nc=nc,
            )
        )

    devices = jax.devices()[:N_CORES]
    mesh = Mesh(np.asarray(devices), ("core",))
    in_specs = tuple(
        PartitionSpec(None) if n in rep_inputs else PartitionSpec("core")
        for n in in_names
    )
    sharded = jax.jit(
        shard_map(
            _body,
            mesh=mesh,
            in_specs=in_specs,
            out_specs=(PartitionSpec("core"),) * len(out_names),
            check_rep=False,
        ),
        keep_unused=True,
    )
    return sharded, in_names, mesh


_R = {}


def _init():
    if "mesh" in _R:
        return
    _R["B"] = _make_runner(_build_agg(True), rep_inputs=("xf",))
    _R["C"] = _make_runner(_build_agg(False), rep_inputs=("xf",))
    mesh = _R["B"][2]
    _R["mesh"] = mesh
    sh = NamedSharding(mesh, PartitionSpec("core"))
    _R["sh"] = sh
    _R["eye"] = jax.device_put(np.tile(np.eye(D, dtype=np.float32), (N_CORES, 1)), sh)
    _R["iota"] = jax.device_put(
        np.tile(np.tile(np.arange(P, dtype=np.float32), (P, 1)), (N_CORES, 1)),
        sh,
    )
    _R["ag"] = jax.jit(
        shard_map(
            lambda s: jax.lax.all_gather(s, "core", axis=0, tiled=True),
            mesh=mesh,
            in_specs=(PartitionSpec("core"),),
            out_specs=PartitionSpec(None),
            check_rep=False,
        )
    )


def _layer(tag, xfull, W, b, edges):
    sharded, in_names, _ = _R[tag]
    sh = _R["sh"]
    feed = {
        "xf": xfull,
        "w": jax.device_put(
            np.tile(np.ascontiguousarray(W, np.float32), (N_CORES, 1)), sh
        ),
        "b": jax.device_put(
            np.tile(np.ascontiguousarray(b, np.float32), N_CORES), sh
        ),
        "e": _R["eye"],
        "iota": _R["iota"],
        "idx": edges[0],
        "dstm": edges[1],
        "nrm": edges[2],
    }
    (out,) = sharded(*[feed[n] for n in in_names])
    return out


def _prep_edges(src_all, dst_all, nrm_all):
    """Bucket edges into [8*NW, P, CPW] padded arrays; None on overflow."""
    order = np.argsort(dst_all, kind="stable")
    gw = (dst_all[order] >> 7).astype(np.int64)
    cnt = np.bincount(gw, minlength=GW)
    if cnt.max() > WSLOTS:
        return None
    starts = np.zeros(GW, np.int64)
    np.cumsum(cnt[:-1], out=starts[1:])
    slot = gw * WSLOTS + (np.arange(len(gw)) - starts[gw])

    idx_p = np.zeros(GW * WSLOTS, np.int32)
    nrm_p = np.zeros(GW * WSLOTS, np.float32)
    dstm_p = np.zeros(GW * WSLOTS, np.uint8)
    idx_p[slot] = src_all[order]
    nrm_p[slot] = nrm_all[order]
    dstm_p[slot] = (dst_all[order] & 127).astype(np.uint8)

    # [GW, CPW, P] -> [GW, P, CPW] (slot within window is c*P + p)
    idx_a = np.ascontiguousarray(idx_p.reshape(GW, CPW, P).transpose(0, 2, 1))
    dstm_a = np.ascontiguousarray(dstm_p.reshape(GW, CPW, P).transpose(0, 2, 1))
    nrm_a = np.ascontiguousarray(
        nrm_p.reshape(GW, CPW, P).transpose(0, 2, 1).astype(BF16)
    )
    sh = _R["sh"]
    return (
        jax.device_put(idx_a, sh),
        jax.device_put(dstm_a, sh),
        jax.device_put(nrm_a, sh),
    )


def _warmup():
    _init()
    z = np.zeros((PADN2, D), BF16)
    zsh = jax.device_put(z, _R["sh"])
    zf = _R["ag"](zsh)
    ze = (
        jax.device_put(np.zeros((GW, P, CPW), np.int32), _R["sh"]),
        jax.device_put(np.zeros((GW, P, CPW), np.uint8), _R["sh"]),
        jax.device_put(np.zeros((GW, P, CPW), BF16), _R["sh"]),
    )
    zw = np.zeros((D, D), np.float32)
    zb = np.zeros((D,), np.float32)
    h = _layer("B", zf, zw, zb, ze)
    hf = _R["ag"](h)
    out = _layer("C", hf, zw, zb, ze)
    np.asarray(out)


try:
    _warmup()
    _DEVICE_OK = True
except Exception as _e:  # pragma: no cover
    print(f"[kernel] device warmup failed ({_e!r}); numpy fallback", file=sys.stderr)
    _DEVICE_OK = False


def _numpy_kernel(x, edge_index, W1, b1, W2, b2):
    import scipy.sparse as sp

    x = np.asarray(x, dtype=np.float32)
    N = x.shape[0]
    loop = np.arange(N, dtype=np.int64)
    src = np.concatenate([np.asarray(edge_index)[0], loop])
    dst = np.concatenate([np.asarray(edge_index)[1], loop])
    deg = np.bincount(dst, minlength=N).astype(np.float32)
    dinv = 1.0 / np.sqrt(deg)
    norm = (dinv[src] * dinv[dst]).astype(np.float32)
    A = sp.csr_matrix((norm, (dst, src)), shape=(N, N), dtype=np.float32)
    h = np.maximum(A @ (x @ np.asarray(W1, np.float32)) + b1, 0.0)
    return (A @ (h @ np.asarray(W2, np.float32)) + b2).astype(np.float32)


def kernel(x, edge_index, W1, b1, W2, b2):
    if not _DEVICE_OK:
        return _numpy_kernel(x, edge_index, W1, b1, W2, b2)

    x = np.asarray(x, dtype=np.float32)
    N = x.shape[0]

    # ship x to the cores first (async); overlap edge prep with transfer
    xp16 = np.zeros((PADN2, D), dtype=BF16)
    xp16[:N] = x
    xsh = jax.device_put(xp16, _R["sh"])
    xfull = _R["ag"](xsh)

    ei = np.asarray(edge_index)
    loop = np.arange(N, dtype=np.int32)
    src_all = np.concatenate([ei[0].astype(np.int32), loop])
    dst_all = np.concatenate([ei[1].astype(np.int32), loop])
    deg = np.bincount(dst_all, minlength=N).astype(np.float32)
    dinv = 1.0 / np.sqrt(deg)
    nrm_all = dinv[src_all] * dinv[dst_all]

    edges = _prep_edges(src_all, dst_all, nrm_all)
    if edges is None:  # window overflow: graph too skewed for CPW
        return _numpy_kernel(x, edge_index, W1, b1, W2, b2)

    h = _layer("B", xfull, W1, np.asarray(b1, np.float32), edges)
    hf = _R["ag"](h)
    out = _layer("C", hf, W2, np.asarray(b2, np.float32), edges)

    return np.asarray(out)[:N].astype(np.float32)


# revision 4
# speedup vs baseline: 57.7990x; 1.6213x over previous
"""2-layer GCN on 8 Trainium2 NeuronCores — fully on-device.

Uses A@(xW) = (A@x)W to aggregate raw features first, then transform
only the local row shard. Per layer: indirect-DMA gather of source rows
from the (replicated) feature table, one-hot selection matrices built
on-device (iota==dstm fused with *norm), PSUM matmul accumulation per
128-row destination window, then shard transform via PE transposes with
bias(+relu) fused in transposed space. Halo exchange = jnp all_gather
between layers (stays on NeuronLink, never crosses the slow host
tunnel). Tunnel traffic: x bf16 in, bucketed edges in, out bf16 back.
"""

import sys
import threading

import numpy as np

for _p in ("/opt/trn_rl_repo",):
    if _p not in sys.path:
        sys.path.insert(0, _p)

N_NODES = 50000
D = 128
P = 128
N_CORES = 8
NP2 = 6272  # rows per core (49 windows x 128)
PADN2 = NP2 * N_CORES  # 50176
NW = 49  # windows per core
CPW = 20  # chunks (of 128 edges) per window
GW = NW * N_CORES  # 392 global windows
WSLOTS = CPW * P  # 2560 edge slots per window

import jax
import ml_dtypes

BF16 = ml_dtypes.bfloat16

try:
    jax.config.update("jax_compilation_cache_dir", "/tmp/jax_bass_cache")
    jax.config.update("jax_persistent_cache_min_entry_size_bytes", -1)
    jax.config.update("jax_persistent_cache_min_compile_time_secs", 0.0)
except Exception:
    pass

import concourse.bass as bass
from concourse import mybir
from concourse.bass2jax import (
    _bass_exec_p,
    install_neuronx_cc_hook,
    partition_id_tensor,
)
from jax.experimental.shard_map import shard_map
from jax.sharding import Mesh, NamedSharding, PartitionSpec

f32 = mybir.dt.float32
bf16 = mybir.dt.bfloat16
i32 = mybir.dt.int32
u16 = mybir.dt.uint16
u8 = mybir.dt.uint8

_AG = {}  # all_gather jits by dtype tag


def _build_agg(relu: bool):
    """yc[NP2,D]bf16 = act((A_local @ xf) @ w + b) for this core's rows.

    xf: [PADN2, D] bf16 full replicated table. Edge bucket arrays are
    [NW, P, CPW]: slot (w, p, c) holds source row idx, dst row offset
    within window (0..127), and the edge norm (0 for padding slots).
    """
    nc = bass.Bass(target_bir_lowering=False)

    xf = nc.dram_tensor("xf", [PADN2, D], bf16, kind="ExternalInput")
    w = nc.dram_tensor("w", [D, D], f32, kind="ExternalInput")
    b = nc.dram_tensor("b", [D], f32, kind="ExternalInput")
    e = nc.dram_tensor("e", [D, D], f32, kind="ExternalInput")
    iota = nc.dram_tensor("iota", [P, P], f32, kind="ExternalInput")
    idx = nc.dram_tensor("idx", [NW, P, CPW], u16, kind="ExternalInput")
    dstm = nc.dram_tensor("dstm", [NW, P, CPW], u8, kind="ExternalInput")
    nrm = nc.dram_tensor("nrm", [NW, P, CPW], bf16, kind="ExternalInput")
    yc = nc.dram_tensor("yc", [NP2, D], bf16, kind="ExternalOutput")

    from contextlib import ExitStack

    with ExitStack() as ctx:
        sem = lambda n: ctx.enter_context(nc.semaphore(n))
        sb = lambda n, s, d: ctx.enter_context(nc.sbuf_tensor(n, s, d))
        ps = lambda n, s, d: ctx.enter_context(nc.psum_tensor(n, s, d))
        ld, gt, sS, mm = sem("ld"), sem("gt"), sem("sS"), sem("mm")
        icv = sem("icv")
        cpZ, tz, cpT, my = sem("cpZ"), sem("tz"), sem("cpT"), sem("my")
        actS, ts2, cpF, st = sem("actS"), sem("ts2"), sem("cpF"), sem("st")
        wsb = sb("wsb", [D, D], f32)
        bsb = sb("bsb", [D, 1], f32)
        esb = sb("esb", [D, D], f32)
        iosb = sb("iosb", [P, P], f32)
        isbu = sb("isbu", [P, CPW], u16)
        isb = sb("isb", [P, CPW], i32)
        dsbu = sb("dsbu", [P, CPW], u8)
        dsbf = sb("dsbf", [P, CPW], f32)
        nsb = sb("nsb", [P, CPW], bf16)
        nsbf = sb("nsbf", [P, CPW], f32)
        gsb0 = sb("gsb0", [P, D], bf16)
        gsb1 = sb("gsb1", [P, D], bf16)
        ssb = sb("ssb", [P, P], bf16)
        zsb = sb("zsb", [P, D], f32)
        ztsb = sb("ztsb", [P, D], f32)
        htsb = sb("htsb", [P, D], f32)
        hob = sb("hob", [P, D], bf16)
        zw = ps("zw", [P, D], f32)
        zt = ps("zt", [P, D], f32)
        psy = ps("psy", [P, D], f32)
        hn = ps("hn", [P, D], f32)
        def full(t, dt_rows=P, dt_cols=D):
            return bass.AP(t, 0, [[dt_cols, dt_rows], [1, dt_cols]])

        ap_w_d = bass.AP(w, 0, [[D, D], [1, D]])
        ap_w_s = full(wsb)
        ap_b_d = bass.AP(b, 0, [[1, D], [1, 1]])
        ap_b_s = bass.AP(bsb, 0, [[1, D], [1, 1]])
        ap_e_d = bass.AP(e, 0, [[D, D], [1, D]])
        ap_e_s = full(esb)
        ap_io_d = bass.AP(iota, 0, [[P, P], [1, P]])
        ap_io_s = full(iosb, P, P)
        ap_xf = bass.AP(xf, 0, [[D, PADN2], [1, D]])

        def ap_wd(t, wi):  # window wi of [NW,P,CPW] dram as [P, CPW]
            return bass.AP(t, wi * P * CPW, [[CPW, P], [1, CPW]])

        def ap_pc(t):  # [P, CPW] sbuf
            return bass.AP(t, 0, [[CPW, P], [1, CPW]])

        def ap_col(t, c):  # column c of [P, CPW] sbuf as [P, 1]
            return bass.AP(t, c, [[CPW, P], [1, 1]])

        def ap_yc(wi):
            return bass.AP(yc, wi * P * D, [[D, P], [1, D]])

        gbufs = [full(gsb0), full(gsb1)]

        with nc.Block() as block:

            @block.gpsimd
            def _(g):
                g.dma_start(ap_w_s, ap_w_d).then_inc(ld, 16)
                g.dma_start(ap_b_s, ap_b_d).then_inc(ld, 16)
                g.dma_start(ap_e_s, ap_e_d).then_inc(ld, 16)
                g.dma_start(ap_io_s, ap_io_d).then_inc(ld, 16)
                for wi in range(NW):
                    if wi >= 1:
                        g.wait_ge(sS, CPW * wi)  # dsbu/dsbf/nsb consumed
                        g.wait_ge(gt, 16 * CPW * wi)  # isb consumed by gathers
                    g.dma_start(ap_pc(isbu), ap_wd(idx, wi)).then_inc(ld, 16)
                    g.dma_start(ap_pc(dsbu), ap_wd(dstm, wi)).then_inc(ld, 16)
                    g.dma_start(ap_pc(nsb), ap_wd(nrm, wi)).then_inc(ld, 16)
                    for c in range(CPW):
                        k = wi * CPW + c
                        if c == 0:
                            g.wait_ge(icv, wi + 1)  # isb widened by vector
                        if k >= 2:
                            g.wait_ge(mm, k - 1)  # gsb[k%2] consumed by PE
                        g.indirect_dma_start(
                            out=gbufs[k % 2],
                            out_offset=None,
                            in_=ap_xf,
                            in_offset=bass.IndirectOffsetOnAxis(
                                ap=ap_col(isb, c), axis=0
                            ),
                        ).then_inc(gt, 16)
                    g.wait_ge(cpF, wi + 1)
                    g.dma_start(ap_yc(wi), full(hob)).then_inc(st, 16)

            @block.vector
            def _(v):
                for wi in range(NW):
                    v.wait_ge(ld, 64 + 48 * (wi + 1))
                    v.tensor_copy(ap_pc(isb), ap_pc(isbu)).then_inc(icv, 1)
                    v.tensor_copy(ap_pc(dsbf), ap_pc(dsbu))
                    v.tensor_copy(ap_pc(nsbf), ap_pc(nsb))
                    for c in range(CPW):
                        k = wi * CPW + c
                        v.wait_ge(mm, k)  # ssb consumed by PE matmul k-1
                        v.tensor_scalar(
                            full(ssb, P, P),
                            ap_io_s,
                            ap_col(dsbf, c),
                            ap_col(nsbf, c),
                            mybir.AluOpType.is_equal,
                            mybir.AluOpType.mult,
                        ).then_inc(sS, 1)
                    if not relu:
                        # plain bias add for the output layer (per-partition
                        # scalar b along fout partitions of psy)
                        v.wait_ge(my, wi + 1)
                        if wi >= 1:
                            v.wait_ge(ts2, wi)  # htsb consumed by PE
                        v.tensor_scalar(
                            full(htsb),
                            full(psy),
                            ap_b_s,
                            None,
                            mybir.AluOpType.add,
                        ).then_inc(actS, 1)
                    v.wait_ge(ts2, wi + 1)
                    if wi >= 1:
                        v.wait_ge(st, 16 * wi)  # hob drained by DMA out
                    v.tensor_copy(full(hob), full(hn)).then_inc(cpF, 1)

            @block.scalar
            def _(s):
                for wi in range(NW):
                    s.wait_ge(mm, (wi + 1) * CPW)
                    if wi >= 1:
                        s.wait_ge(tz, wi)  # zsb consumed by PE transpose
                    s.activation(
                        full(zsb), full(zw), mybir.ActivationFunctionType.Copy
                    ).then_inc(cpZ, 1)
                    s.wait_ge(tz, wi + 1)
                    if wi >= 1:
                        s.wait_ge(my, wi)  # ztsb consumed by PE matmul
                    s.activation(
                        full(ztsb), full(zt), mybir.ActivationFunctionType.Copy
                    ).then_inc(cpT, 1)
                    if relu:
                        s.wait_ge(my, wi + 1)
                        if wi >= 1:
                            s.wait_ge(ts2, wi)  # htsb consumed by PE
                        s.activation(
                            full(htsb),
                            full(psy),
                            mybir.ActivationFunctionType.Relu,
                            bias=ap_b_s,
                        ).then_inc(actS, 1)

            @block.tensor
            def _(t):
                for wi in range(NW):
                    for c in range(CPW):
                        k = wi * CPW + c
                        t.wait_ge(sS, k + 1)
                        t.wait_ge(gt, 16 * (k + 1))
                        if c == 0:
                            t.wait_ge(cpZ, wi)  # zw psum drained by scalar
                        t.matmul(
                            full(zw),
                            full(ssb, P, P),
                            gbufs[k % 2],
                            start=(c == 0),
                            stop=(c == CPW - 1),
                        ).then_inc(mm, 1)
                    t.wait_ge(cpZ, wi + 1)
                    if wi >= 1:
                        t.wait_ge(cpT, wi)  # zt psum drained
                    t.transpose(full(zt), full(zsb), ap_e_s).then_inc(tz, 1)
                    t.wait_ge(cpT, wi + 1)
                    if wi >= 1:
                        t.wait_ge(actS, wi)  # psy drained
                    t.matmul(full(psy), ap_w_s, full(ztsb)).then_inc(my, 1)
                    t.wait_ge(actS, wi + 1)
                    if wi >= 1:
                        t.wait_ge(cpF, wi)  # hn psum drained by vector
                    t.transpose(full(hn), full(htsb), ap_e_s).then_inc(ts2, 1)

    return nc


def _make_runner(nc, rep_inputs=()):
    install_neuronx_cc_hook()
    partition_name = (
        nc.partition_id_tensor.name if nc.partition_id_tensor is not None else None
    )
    in_names, out_names, out_avals = [], [], []
    for alloc in nc.m.functions[0].allocations:
        if not isinstance(alloc, mybir.MemoryLocationSet):
            continue
        name = alloc.memorylocations[0].name
        if alloc.kind == "ExternalInput":
            if name != partition_name:
                in_names.append(name)
        elif alloc.kind == "ExternalOutput":
            out_names.append(name)
            out_avals.append(
                jax.core.ShapedArray(
                    tuple(alloc.tensor_shape), mybir.dt.np(alloc.dtype)
                )
            )
    all_in = list(in_names)
    if partition_name is not None:
        all_in = all_in + [partition_name]
    all_in = tuple(all_in)

    def _body(*args):
        operands = list(args)
        if partition_name is not None:
            operands.append(partition_id_tensor())
        return tuple(
            _bass_exec_p.bind(
                *operands,
                out_avals=tuple(out_avals),
                in_names=all_in,
                out_names=tuple(out_names),
                lowering_input_output_aliases=(),
                sim_require_finite=True,
                sim_require_nnan=True,
                nc=nc,
            )
        )

    devices = jax.devices()[:N_CORES]
    mesh = Mesh(np.asarray(devices), ("core",))
    in_specs = tuple(
        PartitionSpec(None) if n in rep_inputs else PartitionSpec("core")
        for n in in_names
    )
    sharded = jax.jit(
        shard_map(
            _body,
            mesh=mesh,
            in_specs=in_specs,
            out_specs=(PartitionSpec("core"),) * len(out_names),
            check_rep=False,
        ),
        keep_unused=True,
    )
    return sharded, in_names, mesh


_R = {}


def _init():
    if "mesh" in _R:
        return
    _R["B"] = _make_runner(_build_agg(True), rep_inputs=("xf",))
    _R["C"] = _make_runner(_build_agg(False), rep_inputs=("xf",))
    mesh = _R["B"][2]
    _R["mesh"] = mesh
    sh = NamedSharding(mesh, PartitionSpec("core"))
    _R["sh"] = sh
    _R["eye"] = jax.device_put(np.tile(np.eye(D, dtype=np.float32), (N_CORES, 1)), sh)
    _R["iota"] = jax.device_put(
        np.tile(np.tile(np.arange(P, dtype=np.float32), (P, 1)), (N_CORES, 1)),
        sh,
    )
    _R["ag"] = jax.jit(
        shard_map(
            lambda s: jax.lax.all_gather(s, "core", axis=0, tiled=True),
            mesh=mesh,
            in_specs=(PartitionSpec("core"),),
            out_specs=PartitionSpec(None),
            check_rep=False,
        )
    )


def _layer(tag, xfull, W, b, edges):
    sharded, in_names, _ = _R[tag]
    sh = _R["sh"]
    feed = {
        "xf": xfull,
        "w": jax.device_put(
            np.tile(np.ascontiguousarray(W, np.float32), (N_CORES, 1)), sh
        ),
        "b": jax.device_put(
            np.tile(np.ascontiguousarray(b, np.float32), N_CORES), sh
        ),
        "e": _R["eye"],
        "iota": _R["iota"],
        "idx": edges[0],
        "dstm": edges[1],
        "nrm": edges[2],
    }
    (out,) = sharded(*[feed[n] for n in in_names])
    return out


def _prep_edges(src_all, dst_all, nrm_all):
    """Bucket edges into [8*NW, P, CPW] padded arrays; None on overflow."""
    order = np.argsort(dst_all, kind="stable")
    gw = (dst_all[order] >> 7).astype(np.int64)
    cnt = np.bincount(gw, minlength=GW)
    if cnt.max() > WSLOTS:
        return None
    starts = np.zeros(GW, np.int64)
    np.cumsum(cnt[:-1], out=starts[1:])
    pos = np.arange(len(gw)) - starts[gw]
    # slot layout [P, CPW]: edge j of a window lands at (p=j%P, c=j//P)
    slot = gw * WSLOTS + (pos % P) * CPW + pos // P

    idx_p = np.zeros(GW * WSLOTS, np.uint16)
    nrm_p = np.zeros(GW * WSLOTS, np.float32)
    dstm_p = np.zeros(GW * WSLOTS, np.uint8)
    idx_p[slot] = src_all[order]
    nrm_p[slot] = nrm_all[order]
    dstm_p[slot] = (dst_all[order] & 127).astype(np.uint8)

    idx_a = idx_p.reshape(GW, P, CPW)
    dstm_a = dstm_p.reshape(GW, P, CPW)
    nrm_a = nrm_p.reshape(GW, P, CPW).astype(BF16)
    sh = _R["sh"]
    return (
        jax.device_put(idx_a, sh),
        jax.device_put(dstm_a, sh),
        jax.device_put(nrm_a, sh),
    )


def _warmup():
    _init()
    # exercise the full real path once (compiles, dispatch caches, host
    # allocator warm) with a synthetic uniform graph
    rng = np.random.default_rng(0)
    x = rng.standard_normal((N_NODES, D), dtype=np.float32)
    ei = rng.integers(0, N_NODES, size=(2, 800000)).astype(np.int64)
    W = rng.standard_normal((D, D), dtype=np.float32) * 0.09
    b = np.zeros((D,), np.float32)
    _device_kernel(x, ei, W, b, W, b)


try:
    _warmup()
    _DEVICE_OK = True
except Exception as _e:  # pragma: no cover
    print(f"[kernel] device warmup failed ({_e!r}); numpy fallback", file=sys.stderr)
    _DEVICE_OK = False


def _numpy_kernel(x, edge_index, W1, b1, W2, b2):
    import scipy.sparse as sp

    x = np.asarray(x, dtype=np.float32)
    N = x.shape[0]
    loop = np.arange(N, dtype=np.int64)
    src = np.concatenate([np.asarray(edge_index)[0], loop])
    dst = np.concatenate([np.asarray(edge_index)[1], loop])
    deg = np.bincount(dst, minlength=N).astype(np.float32)
    dinv = 1.0 / np.sqrt(deg)
    norm = (dinv[src] * dinv[dst]).astype(np.float32)
    A = sp.csr_matrix((norm, (dst, src)), shape=(N, N), dtype=np.float32)
    h = np.maximum(A @ (x @ np.asarray(W1, np.float32)) + b1, 0.0)
    return (A @ (h @ np.asarray(W2, np.float32)) + b2).astype(np.float32)


def kernel(x, edge_index, W1, b1, W2, b2):
    if not _DEVICE_OK:
        return _numpy_kernel(x, edge_index, W1, b1, W2, b2)
    try:
        return _device_kernel(x, edge_index, W1, b1, W2, b2)
    except Exception as e:  # device/tunnel hiccup: stay correct
        print(f"[kernel] device path failed ({e!r}); numpy fallback", file=sys.stderr)
        return _numpy_kernel(x, edge_index, W1, b1, W2, b2)


def _device_kernel(x, edge_index, W1, b1, W2, b2):

    x = np.asarray(x, dtype=np.float32)
    N = x.shape[0]

    # ship x to the cores first (async); overlap edge prep with transfer
    xp16 = np.zeros((PADN2, D), dtype=BF16)
    xp16[:N] = x
    xsh = jax.device_put(xp16, _R["sh"])
    xfull = _R["ag"](xsh)

    ei = np.asarray(edge_index)
    loop = np.arange(N, dtype=np.int32)
    src_all = np.concatenate([ei[0].astype(np.int32), loop])
    dst_all = np.concatenate([ei[1].astype(np.int32), loop])
    deg = np.bincount(dst_all, minlength=N).astype(np.float32)
    dinv = 1.0 / np.sqrt(deg)
    nrm_all = dinv[src_all] * dinv[dst_all]

    edges = _prep_edges(src_all, dst_all, nrm_all)
    if edges is None:  # window overflow: graph too skewed for CPW
        return _numpy_kernel(x, edge_index, W1, b1, W2, b2)

    h = _layer("B", xfull, W1, np.asarray(b1, np.float32), edges)
    hf = _R["ag"](h)
    out = _layer("C", hf, W2, np.asarray(b2, np.float32), edges)

    return np.asarray(out)[:N].astype(np.float32)
